# revision 1
# baseline (speedup 1.0000x reference)
"""GAT (graph attention) layer on 8 Trainium2 NeuronCores, row-parallel.

out = elu(softmax_row(mask(adj, lrelu(src_i + dst_j))) @ (h @ W))
  with src = (h@W)@a1, dst = (h@W)@a2.

Sharding: each core owns 1024 query rows (rows of the attention matrix);
h/W/a are replicated, adj is row-sharded (host also narrows it to int8 and
pre-transposes h -- pure input marshaling). Inside one core:
  - Wh built from host-pre-transposed hT via PE matmuls (f32r fast path)
  - src for the core's own rows from a host-sliced hTl (full fp32 matmuls)
  - dstb[p, j] = dst_j built during the same hT stream via broadcast-weight
    matmuls (lhsT = w2 replicated along free dim)
  - one fused custom DVE op computes em = lrelu(dstb + src_i) + (adj*BIG - BIG)
  - PE matmul-transposes em chunks into PSUM; the scalar engine applies exp
    while moving PSUM->SBUF (fp16), fusing softmax numerator generation with
    the transpose copy-back
  - aggregation matmul em^T.T @ [Wh | ones] accumulates numerator + row-sums
    per 2048-wide slice; slices are summed into SBUF accumulators so PSUM
    banks rotate freely; normalize by row-sum, elu, DMA out.
"""

import numpy as np

import concourse.bass as bass
import concourse.tile as tile
import concourse.mybir as mybir
from concourse import bacc
from concourse.bass_utils import run_bass_kernel_spmd
from concourse.masks import make_identity

# ---------------- config ----------------
N_NODES, IN_F, OUT_F = 8192, 512, 256
ALPHA = 0.2
BIG = 1.0e30
CORES = 8
R = N_NODES // CORES          # rows per core (1024)
RT = R // 128                 # row-tiles per core (8)
JT = N_NODES // 128           # j-chunks (64)
JS = 2048                     # j-slice for DMA/elementwise
NS = N_NODES // JS            # slices per row-tile (4)
MACRO = 512                   # hT streaming macro tile (nodes)
EM_DT = "f32r"                # "f32r" (precise) or "f16" (faster transposes)

f32 = mybir.dt.float32
f32r = mybir.dt.float32r
f16 = mybir.dt.float16
i8 = mybir.dt.int8

# ---------------- custom DVE op ----------------
_REGISTERED = {}


def _get_custom_op():
    if "op" in _REGISTERED:
        return _REGISTERED["op"]
    import concourse.dve_ops as dve_ops
    from concourse.dve_ops import DveOp, _SUB_OPCODE_FOR_NAME
    from concourse.dve_spec import Spec, Src0, Src1, C0, C1, C2, maxx, lower
    from concourse.dve_uop import DveOpSpec

    name = "LRELU_BIAS_MASK_ANT"
    _t = Src0 + C0
    spec = Spec(
        body=maxx(_t, _t * C2) + (Src1 * C1 - C1),
        reference=lambda in0, in1, s0, s1, imm2: (
            np.maximum(in0 + s0, (in0 + s0) * imm2)
            + (in1.astype(np.float32) * s1 - s1)
        ).astype(np.float32),
    )
    if name not in _SUB_OPCODE_FOR_NAME:
        row = max(_SUB_OPCODE_FOR_NAME.values()) + 1
        _SUB_OPCODE_FOR_NAME[name] = row
        tmp = DveOpSpec(name=name, opcode=row, uops=lower(spec, ver="v3"),
                        rd1_en=True)
        op = DveOp(name, spec, subdim=False, uops_sha={"v3": tmp.sha("v3")})
        dve_ops.OPS.append(op)
        dve_ops.CUSTOM_DVE_SPECS[name] = spec
    else:
        op = next(o for o in dve_ops.OPS if o.name == name)
    _REGISTERED["op"] = op
    return op


# ---------------- kernel builder ----------------
_BUILD_CACHE = {}


def _build_nc(debug=False):
    key = "nc_dbg" if debug else "nc"
    if key in _BUILD_CACHE:
        return _BUILD_CACHE[key]
    OP = _get_custom_op()
    AT = mybir.AluOpType
    AF = mybir.ActivationFunctionType

    nc = bacc.Bacc("TRN2", target_bir_lowering=False, debug=False,
                   num_devices=CORES)

    # hT is declared f32r: host sends raw fp32 bytes; the PE rounds when
    # streaming (measured ~1.5e-4 relative on matmul results).
    hT_ext = nc.dram_tensor("hT", [IN_F, N_NODES], f32r, kind="ExternalInput").ap()
    hTl_ext = nc.dram_tensor("hTl", [IN_F, R], f32, kind="ExternalInput").ap()
    adj_ext = nc.dram_tensor("adj", [R, N_NODES], i8, kind="ExternalInput").ap()
    W_ext = nc.dram_tensor("W", [IN_F, OUT_F], f32, kind="ExternalInput").ap()
    Wt_ext = nc.dram_tensor("Wt", [OUT_F, IN_F], f32, kind="ExternalInput").ap()
    a12_ext = nc.dram_tensor("a12", [OUT_F, 2], f32, kind="ExternalInput").ap()
    out_ext = nc.dram_tensor("out", [R, OUT_F], f32, kind="ExternalOutput").ap()
    if debug:
        dbg_dstb = nc.dram_tensor("dbg_dstb", [128, 512], f32, kind="ExternalOutput").ap()
        dbg_srcl = nc.dram_tensor("dbg_srcl", [128, 8], f32, kind="ExternalOutput").ap()
        dbg_whaug = nc.dram_tensor("dbg_whaug", [128, 4 * (OUT_F + 1)], f32, kind="ExternalOutput").ap()
        dbg_em = nc.dram_tensor("dbg_em", [128, 512], f32, kind="ExternalOutput").ap()
        dbg_agg = nc.dram_tensor("dbg_agg", [128, OUT_F + 1], f32, kind="ExternalOutput").ap()

    KT = IN_F // 128  # 4 contraction tiles

    with tile.TileContext(nc) as tc:
        with tc.tile_pool(name="const", bufs=1) as cpool, \
             tc.tile_pool(name="hT", bufs=3 * KT) as hpool, \
             tc.tile_pool(name="whaug", bufs=1) as wapool, \
             tc.tile_pool(name="small", bufs=1) as spool, \
             tc.tile_pool(name="dstb", bufs=1) as dpool, \
             tc.tile_pool(name="adj", bufs=4) as apool, \
             tc.tile_pool(name="em", bufs=4) as empool, \
             tc.tile_pool(name="pmT", bufs=8) as ptpool, \
             tc.tile_pool(name="outp", bufs=2) as opool, \
             tc.tile_pool(name="dbgp", bufs=1) as dbgpool, \
             tc.tile_pool(name="mm_ps", bufs=3, space="PSUM") as mmps, \
             tc.tile_pool(name="tp_ps", bufs=4, space="PSUM") as tpps, \
             tc.tile_pool(name="mi_ps", bufs=1, space="PSUM") as mips:

            # ---- constants (Wt/a12 first: they gate the dstb chain) ----
            Wtk = []
            for k in range(2):
                t = cpool.tile([128, IN_F], f32, tag=f"Wt{k}")
                nc.sync.dma_start(out=t[:], in_=Wt_ext[k * 128:(k + 1) * 128, :])
                Wtk.append(t)
            a12k = []
            for k in range(2):
                t = cpool.tile([128, 2], f32, tag=f"a12_{k}")
                nc.sync.dma_start(out=t[:], in_=a12_ext[k * 128:(k + 1) * 128, :])
                a12k.append(t)
            Wk, Wkr = [], []
            for k in range(KT):
                t = cpool.tile([128, OUT_F], f32, tag=f"W{k}")
                nc.scalar.dma_start(out=t[:], in_=W_ext[k * 128:(k + 1) * 128, :])
                Wk.append(t)
                tr = cpool.tile([128, OUT_F], f32r, tag=f"Wr{k}")
                nc.vector.tensor_copy(tr[:], t[:])
                Wkr.append(tr)
            id16 = cpool.tile([128, 128], f16, tag="id16")
            make_identity(nc, id16[:])
            em_dt = f16 if EM_DT == "f16" else f32r
            if EM_DT == "f16":
                id_em = id16
            else:
                id_em = cpool.tile([128, 128], f32r, tag="id_em")
                nc.vector.tensor_copy(id_em[:], id16[:])

            # hTl tiles (core's own rows, fp32 exact)
            hTl = []
            for k in range(KT):
                t = cpool.tile([128, R], f32, tag=f"hTl{k}")
                nc.scalar.dma_start(out=t[:], in_=hTl_ext[k * 128:(k + 1) * 128, :])
                hTl.append(t)

            # ---- w1w2[feat, 2] = [W@a1 | W@a2] ----
            w12 = []
            for ftile in range(KT):
                ps = mips.tile([128, 512], f32, tag="mi")
                for k in range(2):
                    nc.tensor.matmul(ps[:, 0:2],
                                     Wtk[k][:, ftile * 128:(ftile + 1) * 128],
                                     a12k[k][:], start=(k == 0), stop=(k == 1))
                t = cpool.tile([128, 2], f32, tag=f"w12_{ftile}")
                nc.vector.tensor_copy(t[:], ps[:, 0:2])
                w12.append(t)
            # w2 replicated along free dim (f32r) for dstb broadcast matmuls
            w2bc = []
            for k in range(KT):
                t = cpool.tile([128, 128], f32r, tag=f"w2bc{k}")
                nc.vector.tensor_copy(t[:], w12[k][:, 1:2].broadcast_to([128, 128]))
                w2bc.append(t)

            # ---- src_local[p, t] = src for the core's own row-tiles (fp32) ----
            src_local = spool.tile([128, 8], f32, tag="src_local")
            slps = mips.tile([128, 512], f32, tag="mi")
            for t in range(RT):
                for k in range(KT):
                    nc.tensor.matmul(slps[:, t:t + 1],
                                     hTl[k][:, t * 128:(t + 1) * 128],
                                     w12[k][:, 0:1],
                                     start=(k == 0), stop=(k == KT - 1))
            nc.vector.tensor_copy(src_local[:], slps[:, 0:8])

            # ---- stream hT: Wh(+fp16 cast) and dstb blocks ----
            whaug = wapool.tile([128, JT * (OUT_F + 1)], f16, tag="whaug")
            wh3 = whaug[:].rearrange("p (c w) -> p c w", w=OUT_F + 1)
            nc.vector.memset(wh3[:, :, OUT_F:OUT_F + 1], 1.0)
            dstb = dpool.tile([128, N_NODES], f32, tag="dstb")

            for im in range(N_NODES // MACRO):
                hkr = []
                for k in range(KT):
                    t = hpool.tile([128, MACRO], f32r, tag="hT")
                    nc.sync.dma_start(
                        out=t[:],
                        in_=hT_ext[k * 128:(k + 1) * 128,
                                   im * MACRO:(im + 1) * MACRO])
                    hkr.append(t)
                # dstb block for this macro
                dps = mips.tile([128, 512], f32, tag="mi")
                for k in range(KT):
                    nc.tensor.matmul(dps[:], w2bc[k][:], hkr[k][:],
                                     start=(k == 0), stop=(k == KT - 1))
                nc.scalar.copy(
                    dstb[:, im * MACRO:(im + 1) * MACRO], dps[:])
                # Wh for the macro's i-tiles
                for it in range(MACRO // 128):
                    g = im * (MACRO // 128) + it
                    sl = slice(it * 128, (it + 1) * 128)
                    wps = mmps.tile([128, OUT_F + 1], f32, tag="mm")
                    for k in range(KT):
                        nc.tensor.matmul(wps[:, 0:OUT_F], hkr[k][:, sl],
                                         Wkr[k][:],
                                         start=(k == 0), stop=(k == KT - 1))
                    nc.scalar.copy(wh3[:, g, 0:OUT_F], wps[:, 0:OUT_F])

            if debug:
                nc.sync.dma_start(out=dbg_srcl[:], in_=src_local[:])
                wtmp = dbgpool.tile([128, 4 * (OUT_F + 1)], f32, tag="wtmp")
                nc.vector.tensor_copy(wtmp[:], whaug[:, 0:4 * (OUT_F + 1)])
                nc.sync.dma_start(out=dbg_whaug[:], in_=wtmp[:])
                nc.sync.dma_start(out=dbg_dstb[:], in_=dstb[:, 0:512])

            # ---- attention: s-outer / t-inner, per-slice psum flushed to SBUF ----
            accs = []
            for t in range(RT):
                a = spool.tile([128, OUT_F + 1], f32, tag=f"acc{t}")
                accs.append(a)
            for s in range(NS):
                for t in range(RT):
                    adj_t = apool.tile([128, JS], i8, tag="adj")
                    nc.scalar.dma_start(
                        out=adj_t[:],
                        in_=adj_ext[t * 128:(t + 1) * 128, s * JS:(s + 1) * JS])
                    em_t = empool.tile([128, JS], em_dt, tag="em")
                    for he in range(2):
                        hs = slice(he * (JS // 2), (he + 1) * (JS // 2))
                        nc.vector._custom_dve(OP, out=em_t[:, hs],
                                              in0=dstb[:, s * JS + he * (JS // 2):
                                                       s * JS + (he + 1) * (JS // 2)],
                                              in1=adj_t[:, hs],
                                              s0=src_local[:, t:t + 1],
                                              s1=BIG, imm2=ALPHA)
                    if debug and t == 0 and s == 0:
                        nc.sync.dma_start(out=dbg_em[:],
                                          in_=em_t[:, 0:512].bitcast(f32) if EM_DT == "f32r" else em_t[:, 0:512])
                    aps = mmps.tile([128, OUT_F + 1], f32, tag="mm")
                    c4s = []
                    for q in range(JS // 512):
                        tp = tpps.tile([128, 512], em_dt, tag="tp")
                        for u in range(4):
                            nc.tensor.matmul(
                                tp[:, u * 128:(u + 1) * 128],
                                em_t[:, (q * 4 + u) * 128:(q * 4 + u + 1) * 128],
                                id_em[:], is_transpose=True,
                                start=(u == 0), stop=(u == 3))
                        c4 = ptpool.tile([128, 512], f16, tag="pmT")
                        nc.scalar.activation(c4[:], tp[:], AF.Exp)
                        c4s.append(c4)
                    nq = JS // 512
                    for q in range(nq):
                        for u in range(4):
                            c = s * (JS // 128) + q * 4 + u
                            nc.tensor.matmul(
                                aps[:],
                                c4s[q][:, u * 128:(u + 1) * 128],
                                wh3[:, c, :],
                                start=(q == 0 and u == 0),
                                stop=(q == nq - 1 and u == 3))
                    if s == 0:
                        nc.vector.tensor_copy(accs[t][:], aps[:])
                    else:
                        nc.vector.tensor_add(accs[t][:], accs[t][:], aps[:])
            for t in range(RT):
                acc = accs[t]
                if debug and t == 0:
                    nc.sync.dma_start(out=dbg_agg[:], in_=acc[:])
                # normalize + elu: out = relu(x) - 1 + exp(min(x, 0)), x = num/den
                rec = opool.tile([128, 1], f32, tag="rec")
                nc.vector.reciprocal(rec[:], acc[:, OUT_F:OUT_F + 1])
                r1 = opool.tile([128, OUT_F], f32, tag="r1")
                nc.vector.tensor_scalar(r1[:], acc[:, 0:OUT_F], rec[:], 0.0,
                                        AT.mult, AT.max)
                xm = opool.tile([128, OUT_F], f32, tag="xm")
                nc.vector.tensor_scalar(xm[:], acc[:, 0:OUT_F], rec[:], 0.0,
                                        AT.mult, AT.min)
                qe = opool.tile([128, OUT_F], f32, tag="qe")
                nc.scalar.activation(qe[:], xm[:], AF.Exp)
                elu = opool.tile([128, OUT_F], f32, tag="elu")
                nc.vector.scalar_tensor_tensor(elu[:], r1[:], -1.0, qe[:],
                                               AT.add, AT.add)
                nc.sync.dma_start(out=out_ext[t * 128:(t + 1) * 128, :],
                                  in_=elu[:])

    nc.finalize()
    _BUILD_CACHE[key] = nc
    return nc


def kernel(h, adj, W, a1, a2):
    h = np.asarray(h, dtype=np.float32)
    W = np.asarray(W, dtype=np.float32)
    a1 = np.asarray(a1, dtype=np.float32)
    a2 = np.asarray(a2, dtype=np.float32)

    nc = _build_nc()

    hT = np.ascontiguousarray(h.T)
    adj8 = (np.asarray(adj) > 0).astype(np.int8)
    Wt = np.ascontiguousarray(W.T)
    a12 = np.ascontiguousarray(np.stack([a1, a2], axis=1))

    in_maps = []
    for c in range(CORES):
        in_maps.append({
            "hT": hT,
            "hTl": np.ascontiguousarray(hT[:, c * R:(c + 1) * R]),
            "adj": adj8[c * R:(c + 1) * R, :],
            "W": W,
            "Wt": Wt,
            "a12": a12,
        })
    res = run_bass_kernel_spmd(nc, in_maps, list(range(CORES)))
    out = np.concatenate([res.results[c]["out"] for c in range(CORES)], axis=0)
    return out



# revision 2
# speedup vs baseline: 1.0627x; 1.0627x over previous
"""GAT layer on 8 TRN2 cores, row-parallel, fp8-centric redesign.

out = elu(softmax_row(mask(adj, lrelu(src_i + dst_j))) @ (h @ W))

Host marshaling (cheap, O(N*F)): src/dst computed exactly on host; per-row
Schraudolph bias B_i baked into the adjacency mask bytes (adjB = adj * B_i).

Device (per core, 1024 query rows):
- Wh = h @ (16W) in bf16 on PE, stored as e4m3 hi + e4m3 residual lo
  (residual via PE accumulate of -I @ hi into the psum, both halves copied
  out by the scalar engine). A 16-valued ones column rides along for the
  softmax denominator.
- scores: ONE fused custom DVE op per j-tile computes, in transposed [j,i]
  layout, int8( max( adj ? lrelu(S(src+dst)) + B_i : 0, 0) ) which IS the
  e4m3 bit pattern of exp(lrelu(logit) - C_i) (Schraudolph-in-fp8, per-row
  shifted; RNE store verified on HW). No transposes, no ACT exp.
- aggregation: fp8 DoubleRow matmuls (2 j-tiles per instruction via 3D APs),
  hi + lo chains accumulating into 8 persistent PSUM accumulators.
- normalize + elu: reciprocal on DVE, the rest on ACT + Pool (DVE is the
  critical path: 64 x 1024-elem custom ops ~= 72us).
"""

import numpy as np
import ml_dtypes

import concourse.bass as bass
import concourse.tile as tile
import concourse.mybir as mybir
from concourse import bacc
from concourse.bass_utils import run_bass_kernel_spmd
from concourse.masks import make_identity

# ---------------- config ----------------
N_NODES, IN_F, OUT_F = 8192, 512, 256
ALPHA = 0.2
CORES = 8
R = N_NODES // CORES          # rows per core (1024)
RT = R // 128                 # i-tiles per core (8)
JT = N_NODES // 128           # j-tiles (64)
NPT = JT // 2                 # j-tile pairs (32)
KT = IN_F // 128              # contraction tiles (4)
MACRO = 512                   # hT macro tile (nodes per DMA)
NM = N_NODES // MACRO         # macros (16)
WCH = OUT_F + 1               # Wh chunk width incl. ones col (257)
S_BITS = 8.0 / float(np.log(2.0))   # e4m3 bits per nat
ONES_VAL_BITS = 0x58          # e4m3 bit pattern of 16.0
Y_TARGET = 110.0              # per-row max score bits

f32 = mybir.dt.float32
f16 = mybir.dt.float16
bf16 = mybir.dt.bfloat16
i8 = mybir.dt.int8
f8e4 = mybir.dt.float8e4

AT = mybir.AluOpType
AF = mybir.ActivationFunctionType

# ---------------- custom DVE op ----------------
_REGISTERED = {}


def _get_custom_op():
    if "op" in _REGISTERED:
        return _REGISTERED["op"]
    import concourse.dve_ops as dve_ops
    from concourse.dve_ops import DveOp, _SUB_OPCODE_FOR_NAME
    from concourse.dve_spec import Spec, Src0, Src1, C0, C1, C2, maxx, select, Zero, lower
    from concourse.dve_uop import DveOpSpec

    name = "SCHRAU_GAT_ANT"
    _t = Src0 + C0
    spec = Spec(
        body=maxx(select(Src1, maxx(_t, _t * C2) + Src1 + C1, Zero), Zero),
        reference=lambda in0, in1, s0, s1, imm2: np.maximum(
            np.where(
                in1 != 0,
                np.maximum(in0 + s0, (in0 + s0) * imm2) + in1.astype(np.float32) + s1,
                0.0,
            ),
            0.0,
        ).astype(np.float32),
    )
    if name not in _SUB_OPCODE_FOR_NAME:
        row = max(_SUB_OPCODE_FOR_NAME.values()) + 1
        _SUB_OPCODE_FOR_NAME[name] = row
        tmp = DveOpSpec(name=name, opcode=row, uops=lower(spec, ver="v3"), rd1_en=True)
        op = DveOp(name, spec, subdim=False, uops_sha={"v3": tmp.sha("v3")})
        dve_ops.OPS.append(op)
        dve_ops.CUSTOM_DVE_SPECS[name] = spec
    else:
        op = next(o for o in dve_ops.OPS if o.name == name)
    _REGISTERED["op"] = op
    return op


# ---------------- kernel builder ----------------
_BUILD_CACHE = {}


def _build_nc():
    if "nc" in _BUILD_CACHE:
        return _BUILD_CACHE["nc"]
    OP = _get_custom_op()

    nc = bacc.Bacc("TRN2", target_bir_lowering=False, debug=False,
                   num_devices=CORES)

    # host-packed inputs
    hP_ext = nc.dram_tensor("hP", [NM * 128, KT * MACRO], bf16,
                            kind="ExternalInput").ap()
    W_ext = nc.dram_tensor("Wp", [128, KT * OUT_F], bf16,
                           kind="ExternalInput").ap()
    adjP_ext = nc.dram_tensor("adjP", [NPT * 128, 2048], i8,
                              kind="ExternalInput").ap()
    srcb_ext = nc.dram_tensor("srcb", [128, R], f16, kind="ExternalInput").ap()
    dstT_ext = nc.dram_tensor("dstT", [128, JT], f32, kind="ExternalInput").ap()
    out_ext = nc.dram_tensor("out", [R, OUT_F], f32, kind="ExternalOutput").ap()

    with tile.TileContext(nc) as tc:
        with tc.tile_pool(name="const", bufs=1) as cpool, \
             tc.tile_pool(name="hP", bufs=4) as hpool, \
             tc.tile_pool(name="wh", bufs=1) as wpool, \
             tc.tile_pool(name="adj", bufs=6) as apool, \
             tc.tile_pool(name="sp", bufs=6) as spool, \
             tc.tile_pool(name="outp", bufs=2) as opool, \
             tc.tile_pool(name="ps", bufs=1, space="PSUM") as pspool:

            # ---- constants ----
            Wt = cpool.tile([128, KT * OUT_F], bf16, tag="Wt")
            nc.sync.dma_start(out=Wt[:], in_=W_ext)
            W3 = Wt[:].rearrange("p (k w) -> p k w", k=KT)
            srcb = cpool.tile([128, R], f16, tag="srcb")
            nc.scalar.dma_start(out=srcb[:], in_=srcb_ext)
            dstT = cpool.tile([128, JT], f32, tag="dstT")
            nc.scalar.dma_start(out=dstT[:], in_=dstT_ext)
            id16 = cpool.tile([128, 128], f16, tag="id16")
            make_identity(nc, id16[:])
            negid8 = cpool.tile([128, 128], i8, tag="negid8")
            nc.scalar.activation(negid8[:].bitcast(f8e4), id16[:], AF.Copy,
                                 scale=-1.0)

            # Wh buffers: per j-tile chunk [Wh(256) | one] as e4m3 bits
            wh_hi = wpool.tile([128, JT * WCH], i8, tag="wh_hi")
            wh_lo = wpool.tile([128, JT * WCH], i8, tag="wh_lo")
            hi3 = wh_hi[:].rearrange("p (g w) -> p g w", w=WCH)
            lo3 = wh_lo[:].rearrange("p (g w) -> p g w", w=WCH)
            nc.vector.memset(hi3[:, :, OUT_F:OUT_F + 1], ONES_VAL_BITS)
            nc.vector.memset(lo3[:, :, OUT_F:OUT_F + 1], 0)

            # ---- interleaved rounds: Wh stream (macro r) + scores (pairs
            # 2r, 2r+1) + fp8 DoubleRow agg (pairs of the previous round, so
            # the wh hi/lo chunks they read are complete).
            # 4 banks of paired accumulators (two 256-wide i-tile accs per
            # 2KB bank), 1 bank of denominators [128, 8]. Banks are zeroed by
            # explicit zero matmuls (start=True zero-regions are whole banks,
            # which would wipe the partner acc mid-stream otherwise).
            accp = [pspool.tile([128, 512], f32, tag=f"b{t}", name=f"accp{t}")
                    for t in range(4)]
            dent = pspool.tile([128, 8], f32, tag="b4", name="dent")
            zrhs = cpool.tile([128, 512], i8, tag="zrhs")
            nc.vector.memset(zrhs[:], 0)
            ones8 = cpool.tile([128, 2], i8, tag="ones8")
            nc.vector.memset(ones8[:], ONES_VAL_BITS)
            for t in range(4):
                nc.tensor.matmul(accp[t][:], negid8[:].bitcast(f8e4),
                                 zrhs[:].bitcast(f8e4), start=True, stop=False,
                                 skip_group_check=True)
            nc.tensor.matmul(dent[:], negid8[:].bitcast(f8e4),
                             zrhs[:, 0:8].bitcast(f8e4), start=True, stop=False,
                             skip_group_check=True)

            def acc_ap(it):
                return accp[it // 2][:, (it % 2) * OUT_F:(it % 2 + 1) * OUT_F]

            def do_scores(pt):
                adjp = apool.tile([128, 2048], i8, tag="adjp", name="adjp")
                nc.sync.dma_start(out=adjp[:],
                                  in_=adjP_ext[pt * 128:(pt + 1) * 128, :])
                spt = spool.tile([128, 2048], i8, tag="spt", name="spt")
                for half in range(2):
                    jt = 2 * pt + half
                    nc.vector._custom_dve(
                        OP,
                        out=spt[:, half * R:(half + 1) * R],
                        in0=srcb[:],
                        in1=adjp[:, half * R:(half + 1) * R],
                        s0=dstT[:, jt:jt + 1],
                        s1=0.0, imm2=ALPHA)
                return spt

            def do_wh_macro(m):
                hp = hpool.tile([128, KT * MACRO], bf16, tag="hp", name="hp")
                nc.sync.dma_start(out=hp[:],
                                  in_=hP_ext[m * 128:(m + 1) * 128, :])
                h3 = hp[:].rearrange("p (k c) -> p k c", k=KT)
                for nt in range(MACRO // 128):
                    g = m * (MACRO // 128) + nt
                    wps = pspool.tile([128, OUT_F], f32, tag=f"b{5 + g % 3}",
                                      name="wps")
                    sl = slice(nt * 128, (nt + 1) * 128)
                    for k in range(KT):
                        nc.tensor.matmul(wps[:], h3[:, k, sl], W3[:, k, :],
                                         start=(k == 0), stop=(k == KT - 1))
                    hi_sl = wh_hi[:, g * WCH:g * WCH + OUT_F].bitcast(f8e4)
                    nc.scalar.activation(hi_sl, wps[:], AF.Copy)
                    nc.tensor.matmul(wps[:], negid8[:].bitcast(f8e4), hi_sl,
                                     start=False, stop=True,
                                     skip_group_check=True)
                    lo_sl = wh_lo[:, g * WCH:g * WCH + OUT_F].bitcast(f8e4)
                    nc.scalar.activation(lo_sl, wps[:], AF.Copy)

            def do_agg(pt, spt):
                sp3 = spt[:].bitcast(f8e4).rearrange("p (two i) -> p two i",
                                                     two=2)
                whh = wh_hi[:, pt * 2 * WCH:(pt + 1) * 2 * WCH].bitcast(f8e4) \
                    .rearrange("p (two w) -> p two w", two=2)
                whl = wh_lo[:, pt * 2 * WCH:(pt + 1) * 2 * WCH].bitcast(f8e4) \
                    .rearrange("p (two w) -> p two w", two=2)
                last = pt == NPT - 1
                for it in range(RT):
                    lhs3 = sp3[:, :, it * 128:(it + 1) * 128]
                    nc.tensor.matmul(
                        acc_ap(it), lhs3, whh[:, :, 0:OUT_F],
                        start=False, stop=False,
                        perf_mode=mybir.MatmulPerfMode.DoubleRow,
                        skip_group_check=True)
                    nc.tensor.matmul(
                        acc_ap(it), lhs3, whl[:, :, 0:OUT_F],
                        start=False, stop=(last and it % 2 == 1),
                        perf_mode=mybir.MatmulPerfMode.DoubleRow,
                        skip_group_check=True)
                    nc.tensor.matmul(
                        dent[:, it:it + 1], lhs3,
                        ones8[:].bitcast(f8e4).rearrange(
                            "p (two w) -> p two w", two=2),
                        start=False, stop=(last and it == RT - 1),
                        perf_mode=mybir.MatmulPerfMode.DoubleRow,
                        skip_group_check=True)

            spts = {}
            for r in range(NM):
                spts[2 * r] = do_scores(2 * r)
                spts[2 * r + 1] = do_scores(2 * r + 1)
                do_wh_macro(r)
                for pt in (2 * r - 2, 2 * r - 1):
                    if pt >= 0:
                        do_agg(pt, spts.pop(pt))
            for pt in (2 * NM - 2, 2 * NM - 1):
                do_agg(pt, spts.pop(pt))

            # ---- phase 4: normalize + elu + out ----
            for it in range(RT):
                rec = opool.tile([128, 1], f32, tag="rec")
                nc.vector.reciprocal(rec[:], dent[:, it:it + 1])
                ar = opool.tile([128, OUT_F], f32, tag="ar")
                nc.scalar.activation(ar[:], acc_ap(it), AF.Copy,
                                     scale=rec[:])
                qe = opool.tile([128, OUT_F], f32, tag="qe")
                nc.scalar.activation(qe[:], ar[:], AF.Exp)
                qm1 = opool.tile([128, OUT_F], f32, tag="qm1")
                nc.vector.tensor_scalar(qm1[:], qe[:], -1.0, 0.0,
                                        AT.add, AT.min)
                elu = opool.tile([128, OUT_F], f32, tag="elu")
                nc.vector.scalar_tensor_tensor(elu[:], ar[:], 0.0, qm1[:],
                                               AT.max, AT.add)
                nc.sync.dma_start(out=out_ext[it * 128:(it + 1) * 128, :],
                                  in_=elu[:])

    nc.finalize()
    _BUILD_CACHE["nc"] = nc
    return nc


def kernel(h, adj, W, a1, a2):
    h = np.asarray(h, dtype=np.float32)
    W = np.asarray(W, dtype=np.float32)
    a1 = np.asarray(a1, dtype=np.float32)
    a2 = np.asarray(a2, dtype=np.float32)
    adj = np.asarray(adj)

    nc = _build_nc()

    # ---- host marshaling ----
    src = (h @ (W @ a1)).astype(np.float32)
    dst = (h @ (W @ a2)).astype(np.float32)
    dstmax = float(dst.max())
    t = src + dstmax
    lr_rowmax = np.maximum(t, t * ALPHA)
    B_i = np.clip(np.round(Y_TARGET - S_BITS * lr_rowmax), 1, 119).astype(np.int8)

    # adjB[i, j] = adj * B_i; transposed + pair-packed per core:
    # adjP rows pt*128+p cover j-tile (2pt, 2pt+1), cols [0:1024 | 1024:2048]
    adjB = (adj > 0).astype(np.int8) * B_i[:, None]          # [i, j]
    adjTB = np.ascontiguousarray(adjB.T)                     # [j, i]

    hT16 = np.ascontiguousarray(h.T).astype(ml_dtypes.bfloat16)   # [512, 8192]
    # hP[m*128+p, k*512+c] = hT16[k*128+p, m*512+c]
    hP = np.ascontiguousarray(
        hT16.reshape(KT, 128, NM, MACRO).transpose(2, 1, 0, 3)
    ).reshape(NM * 128, KT * MACRO)

    W16 = (16.0 * W).astype(ml_dtypes.bfloat16)              # [512, 256]
    Wp = np.ascontiguousarray(
        W16.reshape(KT, 128, OUT_F).transpose(1, 0, 2)
    ).reshape(128, KT * OUT_F)

    dstT = np.ascontiguousarray(
        (S_BITS * dst).astype(np.float32).reshape(JT, 128).T)  # [128, 64]

    in_maps = []
    for c in range(CORES):
        sl = slice(c * R, (c + 1) * R)
        srcb = np.broadcast_to((S_BITS * src[sl]).astype(np.float16),
                               (128, R))
        slab = adjTB[:, sl]                                   # [8192, 1024]
        adjP = np.ascontiguousarray(
            slab.reshape(NPT, 2, 128, R).transpose(0, 2, 1, 3)
        ).reshape(NPT * 128, 2 * R)
        in_maps.append({
            "hP": hP,
            "Wp": Wp,
            "adjP": adjP,
            "srcb": np.ascontiguousarray(srcb),
            "dstT": dstT,
        })
    res = run_bass_kernel_spmd(nc, in_maps, list(range(CORES)))
    out = np.concatenate([res.results[c]["out"] for c in range(CORES)], axis=0)
    return out


# revision 3
# speedup vs baseline: 1.0670x; 1.0041x over previous
"""GAT layer on 8 TRN2 cores, row-parallel, fp8-centric redesign.

out = elu(softmax_row(mask(adj, lrelu(src_i + dst_j))) @ (h @ W))

Host marshaling (cheap, O(N*F)): src/dst computed exactly on host; per-row
Schraudolph bias B_i baked into the adjacency mask bytes (adjB = adj * B_i).

Device (per core, 1024 query rows):
- Wh = h @ (16W) in bf16 on PE, stored as e4m3 hi + e4m3 residual lo
  (residual via PE accumulate of -I @ hi into the psum, both halves copied
  out by the scalar engine). A 16-valued ones column rides along for the
  softmax denominator.
- scores: ONE fused custom DVE op per j-tile computes, in transposed [j,i]
  layout, int8( max( adj ? lrelu(S(src+dst)) + B_i : 0, 0) ) which IS the
  e4m3 bit pattern of exp(lrelu(logit) - C_i) (Schraudolph-in-fp8, per-row
  shifted; RNE store verified on HW). No transposes, no ACT exp.
- aggregation: fp8 DoubleRow matmuls (2 j-tiles per instruction via 3D APs),
  hi + lo chains accumulating into 8 persistent PSUM accumulators.
- normalize + elu: reciprocal on DVE, the rest on ACT + Pool (DVE is the
  critical path: 64 x 1024-elem custom ops ~= 72us).
"""

import numpy as np
import ml_dtypes

import concourse.bass as bass
import concourse.tile as tile
import concourse.mybir as mybir
from concourse import bacc
from concourse.bass_utils import run_bass_kernel_spmd
from concourse.masks import make_identity

# ---------------- config ----------------
N_NODES, IN_F, OUT_F = 8192, 512, 256
ALPHA = 0.2
CORES = 8
R = N_NODES // CORES          # rows per core (1024)
RT = R // 128                 # i-tiles per core (8)
JT = N_NODES // 128           # j-tiles (64)
NPT = JT // 2                 # j-tile pairs (32)
KT = IN_F // 128              # contraction tiles (4)
MACRO = 512                   # hT macro tile (nodes per DMA)
NM = N_NODES // MACRO         # macros (16)
WCH = OUT_F + 1               # Wh chunk width incl. ones col (257)
S_BITS = 8.0 / float(np.log(2.0))   # e4m3 bits per nat
ONES_VAL_BITS = 0x58          # e4m3 bit pattern of 16.0
Y_TARGET = 110.0              # per-row max score bits

f32 = mybir.dt.float32
f16 = mybir.dt.float16
bf16 = mybir.dt.bfloat16
i8 = mybir.dt.int8
f8e4 = mybir.dt.float8e4

AT = mybir.AluOpType
AF = mybir.ActivationFunctionType

# ---------------- custom DVE op ----------------
_REGISTERED = {}


def _get_custom_op():
    if "op" in _REGISTERED:
        return _REGISTERED["op"]
    import concourse.dve_ops as dve_ops
    from concourse.dve_ops import DveOp, _SUB_OPCODE_FOR_NAME
    from concourse.dve_spec import (Spec, Src0, Src1, C0, C1, C2, maxx,
                                    minn, select, Zero, One, lower)
    from concourse.dve_uop import DveOpSpec

    name = "SCHRAU_GAT_ANT"
    _t = Src0 + C0
    spec = Spec(
        body=maxx(select(Src1, maxx(_t, _t * C2) + Src1 + C1, Zero), Zero),
        reference=lambda in0, in1, s0, s1, imm2: np.maximum(
            np.where(
                in1 != 0,
                np.maximum(in0 + s0, (in0 + s0) * imm2) + in1.astype(np.float32) + s1,
                0.0,
            ),
            0.0,
        ).astype(np.float32),
    )
    if name not in _SUB_OPCODE_FOR_NAME:
        row = max(_SUB_OPCODE_FOR_NAME.values()) + 1
        _SUB_OPCODE_FOR_NAME[name] = row
        tmp = DveOpSpec(name=name, opcode=row, uops=lower(spec, ver="v3"), rd1_en=True)
        op = DveOp(name, spec, subdim=False, uops_sha={"v3": tmp.sha("v3")})
        dve_ops.OPS.append(op)
        dve_ops.CUSTOM_DVE_SPECS[name] = spec
    else:
        op = next(o for o in dve_ops.OPS if o.name == name)
    _REGISTERED["op"] = op

    name2 = "ELU_COMBINE_ANT"
    spec2 = Spec(
        body=maxx(Src0, Zero) + minn(Src1 - One, Zero),
        reference=lambda in0, in1, s0, s1, imm2: (
            np.maximum(in0, 0.0) + np.minimum(in1.astype(np.float32) - 1.0, 0.0)
        ).astype(np.float32),
    )
    if name2 not in _SUB_OPCODE_FOR_NAME:
        row2 = max(_SUB_OPCODE_FOR_NAME.values()) + 1
        _SUB_OPCODE_FOR_NAME[name2] = row2
        tmp2 = DveOpSpec(name=name2, opcode=row2, uops=lower(spec2, ver="v3"),
                         rd1_en=True)
        op2 = DveOp(name2, spec2, subdim=False, uops_sha={"v3": tmp2.sha("v3")})
        dve_ops.OPS.append(op2)
        dve_ops.CUSTOM_DVE_SPECS[name2] = spec2
    else:
        op2 = next(o for o in dve_ops.OPS if o.name == name2)
    _REGISTERED["op2"] = op2
    return op


# ---------------- kernel builder ----------------
_BUILD_CACHE = {}


def _build_nc():
    if "nc" in _BUILD_CACHE:
        return _BUILD_CACHE["nc"]
    OP = _get_custom_op()
    OP2 = _REGISTERED["op2"]

    nc = bacc.Bacc("TRN2", target_bir_lowering=False, debug=False,
                   num_devices=CORES)

    # host-packed inputs
    hP_ext = nc.dram_tensor("hP", [NM * 128, KT * MACRO], bf16,
                            kind="ExternalInput").ap()
    W_ext = nc.dram_tensor("Wp", [128, KT * OUT_F], bf16,
                           kind="ExternalInput").ap()
    adjP_ext = nc.dram_tensor("adjP", [NPT * 128, 2048], i8,
                              kind="ExternalInput").ap()
    srcb_ext = nc.dram_tensor("srcb", [128, R], f16, kind="ExternalInput").ap()
    dstT_ext = nc.dram_tensor("dstT", [128, JT], f32, kind="ExternalInput").ap()
    out_ext = nc.dram_tensor("out", [R, OUT_F], f32, kind="ExternalOutput").ap()

    with tile.TileContext(nc) as tc:
        with tc.tile_pool(name="const", bufs=1) as cpool, \
             tc.tile_pool(name="hP", bufs=4) as hpool, \
             tc.tile_pool(name="wh", bufs=1) as wpool, \
             tc.tile_pool(name="adj", bufs=6) as apool, \
             tc.tile_pool(name="sp", bufs=6) as spool, \
             tc.tile_pool(name="outp", bufs=2) as opool, \
             tc.tile_pool(name="ps", bufs=1, space="PSUM") as pspool:

            # ---- constants ----
            Wt = cpool.tile([128, KT * OUT_F], bf16, tag="Wt")
            nc.sync.dma_start(out=Wt[:], in_=W_ext)
            W3 = Wt[:].rearrange("p (k w) -> p k w", k=KT)
            srcb = cpool.tile([128, R], f16, tag="srcb")
            nc.scalar.dma_start(out=srcb[:], in_=srcb_ext)
            dstT = cpool.tile([128, JT], f32, tag="dstT")
            nc.scalar.dma_start(out=dstT[:], in_=dstT_ext)
            id16 = cpool.tile([128, 128], f16, tag="id16")
            make_identity(nc, id16[:])
            negid8 = cpool.tile([128, 128], i8, tag="negid8")
            nc.scalar.activation(negid8[:].bitcast(f8e4), id16[:], AF.Copy,
                                 scale=-1.0)

            # Wh buffers: per j-tile chunk [Wh(256) | one] as e4m3 bits
            wh_hi = wpool.tile([128, JT * WCH], i8, tag="wh_hi")
            wh_lo = wpool.tile([128, JT * WCH], i8, tag="wh_lo")
            hi3 = wh_hi[:].rearrange("p (g w) -> p g w", w=WCH)
            lo3 = wh_lo[:].rearrange("p (g w) -> p g w", w=WCH)
            nc.vector.memset(hi3[:, :, OUT_F:OUT_F + 1], ONES_VAL_BITS)
            nc.vector.memset(lo3[:, :, OUT_F:OUT_F + 1], 0)

            # ---- interleaved rounds: Wh stream (macro r) + scores (pairs
            # 2r, 2r+1) + fp8 DoubleRow agg (pairs of the previous round, so
            # the wh hi/lo chunks they read are complete).
            # 4 banks of paired accumulators (two 256-wide i-tile accs per
            # 2KB bank), 1 bank of denominators [128, 8]. Banks are zeroed by
            # explicit zero matmuls (start=True zero-regions are whole banks,
            # which would wipe the partner acc mid-stream otherwise).
            accp = [pspool.tile([128, 512], f32, tag=f"b{t}", name=f"accp{t}")
                    for t in range(4)]
            dent = pspool.tile([128, 8], f32, tag="b4", name="dent")
            zrhs = cpool.tile([128, 512], i8, tag="zrhs")
            nc.vector.memset(zrhs[:], 0)
            ones8 = cpool.tile([128, 2], i8, tag="ones8")
            nc.vector.memset(ones8[:], ONES_VAL_BITS)
            for t in range(4):
                nc.tensor.matmul(accp[t][:], negid8[:].bitcast(f8e4),
                                 zrhs[:].bitcast(f8e4), start=True, stop=False,
                                 skip_group_check=True)
            nc.tensor.matmul(dent[:], negid8[:].bitcast(f8e4),
                             zrhs[:, 0:8].bitcast(f8e4), start=True, stop=False,
                             skip_group_check=True)

            def acc_ap(it):
                return accp[it // 2][:, (it % 2) * OUT_F:(it % 2 + 1) * OUT_F]

            def do_scores(pt):
                adjp = apool.tile([128, 2048], i8, tag="adjp", name="adjp")
                nc.sync.dma_start(out=adjp[:],
                                  in_=adjP_ext[pt * 128:(pt + 1) * 128, :])
                spt = spool.tile([128, 2048], i8, tag="spt", name="spt")
                for half in range(2):
                    jt = 2 * pt + half
                    nc.vector._custom_dve(
                        OP,
                        out=spt[:, half * R:(half + 1) * R],
                        in0=srcb[:],
                        in1=adjp[:, half * R:(half + 1) * R],
                        s0=dstT[:, jt:jt + 1],
                        s1=0.0, imm2=ALPHA)
                return spt

            def do_wh_macro(m):
                hp = hpool.tile([128, KT * MACRO], bf16, tag="hp", name="hp")
                nc.sync.dma_start(out=hp[:],
                                  in_=hP_ext[m * 128:(m + 1) * 128, :])
                h3 = hp[:].rearrange("p (k c) -> p k c", k=KT)
                for nt in range(MACRO // 128):
                    g = m * (MACRO // 128) + nt
                    wps = pspool.tile([128, OUT_F], f32, tag=f"b{5 + g % 3}",
                                      name="wps")
                    sl = slice(nt * 128, (nt + 1) * 128)
                    for k in range(KT):
                        nc.tensor.matmul(wps[:], h3[:, k, sl], W3[:, k, :],
                                         start=(k == 0), stop=(k == KT - 1))
                    hi_sl = wh_hi[:, g * WCH:g * WCH + OUT_F].bitcast(f8e4)
                    nc.scalar.activation(hi_sl, wps[:], AF.Copy)
                    nc.tensor.matmul(wps[:], negid8[:].bitcast(f8e4), hi_sl,
                                     start=False, stop=True,
                                     skip_group_check=True)
                    lo_sl = wh_lo[:, g * WCH:g * WCH + OUT_F].bitcast(f8e4)
                    nc.scalar.activation(lo_sl, wps[:], AF.Copy)

            def do_agg(pt, spt):
                sp3 = spt[:].bitcast(f8e4).rearrange("p (two i) -> p two i",
                                                     two=2)
                whh = wh_hi[:, pt * 2 * WCH:(pt + 1) * 2 * WCH].bitcast(f8e4) \
                    .rearrange("p (two w) -> p two w", two=2)
                whl = wh_lo[:, pt * 2 * WCH:(pt + 1) * 2 * WCH].bitcast(f8e4) \
                    .rearrange("p (two w) -> p two w", two=2)
                last = pt == NPT - 1
                for it in range(RT):
                    lhs3 = sp3[:, :, it * 128:(it + 1) * 128]
                    nc.tensor.matmul(
                        acc_ap(it), lhs3, whh[:, :, 0:OUT_F],
                        start=False, stop=False,
                        perf_mode=mybir.MatmulPerfMode.DoubleRow,
                        skip_group_check=True)
                    nc.tensor.matmul(
                        acc_ap(it), lhs3, whl[:, :, 0:OUT_F],
                        start=False, stop=(last and it % 2 == 1),
                        perf_mode=mybir.MatmulPerfMode.DoubleRow,
                        skip_group_check=True)
                    nc.tensor.matmul(
                        dent[:, it:it + 1], lhs3,
                        ones8[:].bitcast(f8e4).rearrange(
                            "p (two w) -> p two w", two=2),
                        start=False, stop=(last and it == RT - 1),
                        perf_mode=mybir.MatmulPerfMode.DoubleRow,
                        skip_group_check=True)

            spts = {}
            for r in range(NM):
                spts[2 * r] = do_scores(2 * r)
                spts[2 * r + 1] = do_scores(2 * r + 1)
                do_wh_macro(r)
                for pt in (2 * r - 2, 2 * r - 1):
                    if pt >= 0:
                        do_agg(pt, spts.pop(pt))
            for pt in (2 * NM - 2, 2 * NM - 1):
                do_agg(pt, spts.pop(pt))

            # ---- phase 4: normalize + elu + out ----
            # Per bank-pair: rec (DVE), two scaled copies (ACT + DVE), Exp
            # (ACT), fused elu-combine (custom DVE), one paired out DMA.
            for tp_ in range(4):
                it0, it1 = 2 * tp_, 2 * tp_ + 1
                rec = opool.tile([128, 2], f32, tag=f"rec{tp_}", name="rec")
                nc.vector.reciprocal(rec[:], dent[:, it0:it0 + 2])
                ar = opool.tile([128, 2 * OUT_F], f32, tag=f"ar{tp_}",
                                name="ar")
                nc.scalar.activation(ar[:, 0:OUT_F], acc_ap(it0), AF.Copy,
                                     scale=rec[:, 0:1])
                nc.vector.tensor_scalar(ar[:, OUT_F:2 * OUT_F], acc_ap(it1),
                                        rec[:, 1:2], 0.0,
                                        AT.mult, AT.bypass)
                qe = opool.tile([128, 2 * OUT_F], f32, tag=f"qe{tp_}",
                                name="qe")
                nc.scalar.activation(qe[:], ar[:], AF.Exp)
                elu = opool.tile([128, 2 * OUT_F], f32, tag=f"elu{tp_}",
                                name="elu")
                nc.vector._custom_dve(OP2, out=elu[:], in0=ar[:], in1=qe[:],
                                      s0=0.0, s1=0.0, imm2=0.0)
                nc.sync.dma_start(
                    out=out_ext[it0 * 128:(it0 + 2) * 128, :].rearrange(
                        "(two p) w -> p two w", two=2),
                    in_=elu[:].rearrange("p (two w) -> p two w", two=2))

    nc.finalize()
    _BUILD_CACHE["nc"] = nc
    return nc


def kernel(h, adj, W, a1, a2):
    h = np.asarray(h, dtype=np.float32)
    W = np.asarray(W, dtype=np.float32)
    a1 = np.asarray(a1, dtype=np.float32)
    a2 = np.asarray(a2, dtype=np.float32)
    adj = np.asarray(adj)

    nc = _build_nc()

    # ---- host marshaling ----
    src = (h @ (W @ a1)).astype(np.float32)
    dst = (h @ (W @ a2)).astype(np.float32)
    dstmax = float(dst.max())
    t = src + dstmax
    lr_rowmax = np.maximum(t, t * ALPHA)
    B_i = np.clip(np.round(Y_TARGET - S_BITS * lr_rowmax), 1, 119).astype(np.int8)

    # adjB[i, j] = adj * B_i; transposed + pair-packed per core:
    # adjP rows pt*128+p cover j-tile (2pt, 2pt+1), cols [0:1024 | 1024:2048]
    adjB = (adj > 0).astype(np.int8) * B_i[:, None]          # [i, j]
    adjTB = np.ascontiguousarray(adjB.T)                     # [j, i]

    hT16 = np.ascontiguousarray(h.T).astype(ml_dtypes.bfloat16)   # [512, 8192]
    # hP[m*128+p, k*512+c] = hT16[k*128+p, m*512+c]
    hP = np.ascontiguousarray(
        hT16.reshape(KT, 128, NM, MACRO).transpose(2, 1, 0, 3)
    ).reshape(NM * 128, KT * MACRO)

    W16 = (16.0 * W).astype(ml_dtypes.bfloat16)              # [512, 256]
    Wp = np.ascontiguousarray(
        W16.reshape(KT, 128, OUT_F).transpose(1, 0, 2)
    ).reshape(128, KT * OUT_F)

    dstT = np.ascontiguousarray(
        (S_BITS * dst).astype(np.float32).reshape(JT, 128).T)  # [128, 64]

    in_maps = []
    for c in range(CORES):
        sl = slice(c * R, (c + 1) * R)
        srcb = np.broadcast_to((S_BITS * src[sl]).astype(np.float16),
                               (128, R))
        slab = adjTB[:, sl]                                   # [8192, 1024]
        adjP = np.ascontiguousarray(
            slab.reshape(NPT, 2, 128, R).transpose(0, 2, 1, 3)
        ).reshape(NPT * 128, 2 * R)
        in_maps.append({
            "hP": hP,
            "Wp": Wp,
            "adjP": adjP,
            "srcb": np.ascontiguousarray(srcb),
            "dstT": dstT,
        })
    res = run_bass_kernel_spmd(nc, in_maps, list(range(CORES)))
    out = np.concatenate([res.results[c]["out"] for c in range(CORES)], axis=0)
    return out
